# revision 8
# baseline (speedup 1.0000x reference)
"""KGAN 2-hop knowledge-graph attention on 8 Trainium2 NeuronCores.

Strategy (data-parallel over batch; entity table replicated):
  - Each core: BL=32 batches, per hop 512 groups (b, r) x 32 memories
    = 16384 entity-row gathers for h and for t.
  - Entity rows are fetched with dma_gather (int16 indices) by splitting
    the 500k-row table into 16 buckets of 32768 rows; slots are
    bucket-sorted on host, so rows land "scrambled" in SBUF staging.
  - A SBUF->SBUF dma_scatter_add (dest pre-zeroed; unique destinations,
    CCE-add acts as copy) regroups rows into the (group-partition,
    memory-column) layout. Item rows (v) ride along hop-0's h stream.
  - Relation modulation via a host-built 256-row relation-PAIR table
    ([rel[a] | rel[b]] rows, 512B) gathered directly in grouped order.
  - SWDGE work is spread over 4 Q7 queue-pairs; every Pool-engine DMA
    instruction takes queue (1,2,3,0)[i % 4] in issue order (the tile
    scheduler's 8 DMA-completion lanes round-robin and lock to a queue;
    queue-0 instructions block the Pool sequencer, so they come last in
    each rotation).
  - scores = reduce_d(h * relpair * v); softmax over m on ACT with fused
    exp+sum (max subtraction skipped: |scores| << 1 for 0.02-scale
    embeddings); out = reduce_m(t * p) / sum.
"""

import numpy as np

N_ENT = 500001
B = 256
R = 16
D = 64
M = 32
HOPS = 2
NCORES = 8
BL = B // NCORES          # 32 local batches per core
G = BL * R                # 512 groups (b, r) per hop per core
TPH = G // 128            # 4 tiles of 128 groups per hop
NBUK = 16                 # entity-id buckets of 32768 rows
BUKCAP = 1280             # fixed per-bucket capacity (mean 1024/1056)
SLOTS = NBUK * BUKCAP     # 20480 staged slots per (hop, tensor)
HSLOTS = SLOTS // 2       # 10240 per half (8 buckets)
NPAIR = M // 2            # 16 relation pairs per group
VCOL = TPH * M            # v cols 128..131
TRASH = VCOL + TPH        # trash col 132
DCOLS = TRASH + 2         # 134 dest columns (pair-even)
GCHUNK = 640              # gather idxs per instruction (41 descs/engine)
SCH = (512, 512, 256)     # scatter chunk split per bucket (<=65 descs)
import os
QMAP = tuple(int(x) for x in os.environ.get("KQMAP", "1,2,3,0").split(","))

_NC = None


def _build_program():
    import concourse.bacc as bacc
    import concourse.tile as tile
    from concourse import mybir

    dt = mybir.dt
    f32 = dt.float32
    i16 = dt.int16
    Alu = mybir.AluOpType
    Axis = mybir.AxisListType

    nc = bacc.Bacc("TRN2", debug=False, num_devices=NCORES, num_swdge_queues=4)

    ent = nc.dram_tensor("entity", (NBUK * 32768, D), f32, kind="ExternalInput").ap()
    relp = nc.dram_tensor("relpair", (R * R, 2 * D), f32, kind="ExternalInput").ap()
    gi = nc.dram_tensor("gi", (128, HOPS * 2 * (SLOTS // 16)), i16, kind="ExternalInput").ap()
    si = nc.dram_tensor("si", (128, HOPS * 2 * (SLOTS // 16)), i16, kind="ExternalInput").ap()
    ri = nc.dram_tensor("ri", (128, HOPS * TPH * (2048 // 16)), i16, kind="ExternalInput").ap()
    out = nc.dram_tensor("out", (HOPS, TPH, 128, D), f32, kind="ExternalOutput").ap()

    qctr = [0]

    def q():
        v = QMAP[qctr[0] % 4]
        qctr[0] += 1
        return v

    with tile.TileContext(nc) as tc:
        with (
            tc.tile_pool(name="idx", bufs=1) as idxp,
            tc.tile_pool(name="stg", bufs=2) as stgp,
            tc.tile_pool(name="grp", bufs=2) as grpp,
            tc.tile_pool(name="rel", bufs=4) as rpool,
            tc.tile_pool(name="sml", bufs=2) as sml,
            tc.tile_pool(name="vv", bufs=1) as vvp,
        ):
            gi_sb = idxp.tile([128, HOPS * 2 * (SLOTS // 16)], i16)
            si_sb = idxp.tile([128, HOPS * 2 * (SLOTS // 16)], i16)
            ri_sb = idxp.tile([128, HOPS * TPH * (2048 // 16)], i16)
            nc.sync.dma_start(out=gi_sb, in_=gi)
            nc.sync.dma_start(out=si_sb, in_=si)
            nc.sync.dma_start(out=ri_sb, in_=ri)

            v_sb = vvp.tile([128, TPH * D], f32)

            for hop in range(HOPS):
                # relation pair rows, grouped order, per tile j
                rel_j = []
                for j in range(TPH):
                    rt = rpool.tile([128, NPAIR * 2 * D], f32, tag="rel", name=f"rel{hop}{j}")
                    rel_j.append(rt)
                    for half in range(2):
                        base = (hop * TPH + j) * 128 + half * 64
                        nc.gpsimd.dma_gather(
                            out_ap=rt.rearrange("p (c e) -> p c e", e=2 * D)[
                                :, half * 8 : (half + 1) * 8
                            ],
                            in_ap=relp,
                            idxs_ap=ri_sb[:, base : base + 64],
                            num_idxs=1024,
                            num_idxs_reg=1024,
                            elem_size=2 * D,
                            queue_num=q(),
                        )

                grp = {}
                for tens in range(2):  # 0 = h (+v on hop 0), 1 = t
                    g_t = grpp.tile([128, DCOLS * D], f32, tag="grp", name=f"g{hop}{tens}")
                    grp[tens] = g_t
                    nc.vector.memset(g_t, 0.0)
                    half_elems = (DCOLS // 2) * D
                    gA = g_t[:, 0:half_elems].rearrange("p (c e) -> p c e", e=D)
                    gB = g_t[:, half_elems : 2 * half_elems].rearrange(
                        "p (c e) -> p c e", e=D
                    )
                    iof = (hop * 2 + tens) * (SLOTS // 16)
                    for half in range(2):
                        stg = stgp.tile(
                            [128, (HSLOTS // 128) * D], f32, tag="stg",
                            name=f"s{hop}{tens}{half}",
                        )
                        s3 = stg.rearrange("p (c e) -> p c e", e=D)
                        for kk in range(NBUK // 2):  # 8 buckets per half
                            k = half * 8 + kk
                            for gg in range(2):  # two 640-idx gathers/bucket
                                nc.gpsimd.dma_gather(
                                    out_ap=s3[:, kk * 10 + gg * 5 : kk * 10 + (gg + 1) * 5],
                                    in_ap=ent[k * 32768 : (k + 1) * 32768],
                                    idxs_ap=gi_sb[
                                        :, iof + k * 80 + gg * 40 : iof + k * 80 + (gg + 1) * 40
                                    ],
                                    num_idxs=GCHUNK,
                                    num_idxs_reg=GCHUNK,
                                    elem_size=D,
                                    queue_num=q(),
                                )
                            off = 0  # scatter chunks within bucket
                            for n_s in SCH:
                                nc.gpsimd.dma_scatter_add(
                                    out_ap=gA,
                                    in_ap=s3[
                                        :, kk * 10 + off // 128 : kk * 10 + (off + n_s) // 128
                                    ],
                                    idxs_ap=si_sb[
                                        :,
                                        iof + k * 80 + off // 16 : iof + k * 80 + (off + n_s) // 16,
                                    ],
                                    num_idxs=n_s,
                                    num_idxs_reg=n_s,
                                    elem_size=D,
                                    queue_num=q(),
                                    sbuf_tokens_per_rank=128,
                                    parity_reg=0,
                                    out_ap_other=gB,
                                )
                                off += n_s

                if hop == 0:
                    gv = grp[0].rearrange(
                        "p (half c e) -> p half c e", half=2, e=D
                    )[:, :, 64:66, :]
                    nc.scalar.copy(
                        out=v_sb.rearrange("p (c half e) -> p half c e", half=2, e=D),
                        in_=gv,
                    )

                gv0 = grp[0].rearrange("p (half c e) -> p half c e", half=2, e=D)
                gv1 = grp[1].rearrange("p (half c e) -> p half c e", half=2, e=D)
                for j in range(TPH):
                    hj = gv0[:, :, j * 16 : (j + 1) * 16, :]
                    tj = gv1[:, :, j * 16 : (j + 1) * 16, :]
                    rj = rel_j[j].rearrange(
                        "p (c half e) -> p half c e", half=2, e=D
                    )

                    hr = sml.tile([128, M * D], f32, tag="hr", bufs=1)
                    nc.vector.tensor_tensor(
                        out=hr.rearrange("p (half c e) -> p half c e", half=2, e=D),
                        in0=hj,
                        in1=rj,
                        op=Alu.mult,
                    )
                    hrv = sml.tile([128, M * D], f32, tag="hrv", bufs=1)
                    v_b = (
                        v_sb[:, j * D : (j + 1) * D]
                        .rearrange("p (o d) -> p o d", o=1)
                        .to_broadcast([128, M, D])
                    )
                    nc.vector.tensor_tensor(out=hrv, in0=hr, in1=v_b, op=Alu.mult)

                    scores = sml.tile([128, M], f32, tag="sc")
                    nc.vector.tensor_reduce(
                        out=scores,
                        in_=hrv.rearrange("p (m d) -> p m d", d=D),
                        axis=Axis.X,
                        op=Alu.add,
                    )
                    sexp = sml.tile([128, M], f32, tag="se")
                    ssum = sml.tile([128, 1], f32, tag="ss")
                    nc.scalar.activation(
                        out=sexp,
                        in_=scores,
                        func=mybir.ActivationFunctionType.Exp,
                        scale=1.0,
                        accum_out=ssum,
                    )
                    srec = sml.tile([128, 1], f32, tag="sr")
                    nc.vector.reciprocal(out=srec, in_=ssum)

                    tp = sml.tile([128, M * D], f32, tag="tp", bufs=1)
                    se_b = (
                        sexp.rearrange("p (m o) -> p m o", o=1)
                        .to_broadcast([128, M, D])
                        .rearrange("p (half c) e -> p half c e", half=2)
                    )
                    nc.vector.tensor_tensor(
                        out=tp.rearrange("p (half c e) -> p half c e", half=2, e=D),
                        in0=tj,
                        in1=se_b,
                        op=Alu.mult,
                    )
                    outr = sml.tile([128, D], f32, tag="or")
                    nc.vector.tensor_reduce(
                        out=outr,
                        in_=tp.rearrange("p (m d) -> p d m", d=D),
                        axis=Axis.X,
                        op=Alu.add,
                    )
                    out_t = sml.tile([128, D], f32, tag="ot")
                    nc.vector.tensor_scalar_mul(out=out_t, in0=outr, scalar1=srec)
                    nc.sync.dma_start(out=out[hop, j], in_=out_t)

    # Re-derive queue_num from the scheduler-assigned DMASW lane so that
    # each completion-sem lane is used by exactly one SWDGE queue (the Q7
    # shadow-semaphore trackers are per-queue). Lanes 11..18 = DMASW0..7.
    for inst in nc.inst_map.values():
        proc = getattr(inst, "bass_scheduled_proc", None)
        if proc is not None and 11 <= proc <= 18 and hasattr(inst, "queue_num"):
            inst.queue_num = QMAP[(proc - 11) % 4]

    nc.compile()
    return nc


def _get_nc():
    global _NC
    if _NC is None:
        _NC = _build_program()
    return _NC


def _prep_shared(entity_emb, relation_emb):
    ent = np.asarray(entity_emb, dtype=np.float32)
    entp = np.zeros((NBUK * 32768, D), np.float32)
    entp[:N_ENT] = ent
    rel = np.asarray(relation_emb, dtype=np.float32)
    relpair = np.concatenate(
        [np.repeat(rel, R, axis=0), np.tile(rel, (R, 1))], axis=1
    )  # row a*R+b = [rel[a] | rel[b]]
    return np.ascontiguousarray(entp), np.ascontiguousarray(relpair)


def _wrap16(idx):
    """flat list (len % 16 == 0) -> [128, len/16] int16 (replicated x8)."""
    idx = np.asarray(idx, np.int16)
    return np.ascontiguousarray(np.tile(idx.reshape(-1, 16).T, (8, 1)))


def _bucketize(ids, codes):
    """Bucket-sort slots by entity-id >> 15 with fixed per-bucket capacity.

    Returns (gidx[SLOTS], scode[SLOTS]) int16."""
    gidx = np.zeros(SLOTS, np.int32)
    scode = np.zeros(SLOTS, np.int32)
    buk = ids >> 15
    order = np.argsort(buk, kind="stable")
    counts = np.bincount(buk, minlength=NBUK)
    if counts.max() > BUKCAP:
        raise ValueError(f"bucket overflow: {counts.max()} > {BUKCAP}")
    pos = 0
    for b in range(NBUK):
        n = counts[b]
        sl = order[pos : pos + n]
        base = b * BUKCAP
        gidx[base : base + n] = ids[sl] - (b << 15)
        scode[base : base + n] = codes[sl]
        pad = BUKCAP - n
        if pad:
            # pads gather local row 0 (harmless) and scatter to trash
            scode[base + n : base + BUKCAP] = TRASH * 128 + (np.arange(pad) % 128)
        pos += n
    return gidx.astype(np.int16), scode.astype(np.int16)


def _prep_core(items32, mh_c, mr_c, mt_c):
    """Per-core host index prep. Group g = j*128 + p (tile j, partition p)."""
    gi_host = np.zeros((HOPS, 2, 128, SLOTS // 16), np.int16)
    si_host = np.zeros((HOPS, 2, 128, SLOTS // 16), np.int16)
    ri_host = np.zeros((HOPS, TPH, 128, 2048 // 16), np.int16)
    # (reassembled into [128, cols] below)

    g_of = np.arange(G * M) // M
    m_of = np.arange(G * M) % M
    codes_base = ((g_of // 128) * M + m_of) * 128 + (g_of % 128)

    for hop in range(HOPS):
        for tens in range(2):
            src = (mh_c if tens == 0 else mt_c)[hop]  # [G, M]
            ids = src.reshape(-1).astype(np.int64)
            codes = codes_base
            if hop == 0 and tens == 0:
                gg = np.arange(G)
                v_codes = (VCOL + gg // 128) * 128 + (gg % 128)
                v_ids = items32[(gg // R)].astype(np.int64)
                ids = np.concatenate([ids, v_ids])
                codes = np.concatenate([codes_base, v_codes])
            gidx, scode = _bucketize(ids, codes)
            gi_host[hop, tens] = _wrap16(gidx)
            si_host[hop, tens] = _wrap16(scode)

        mr_h = mr_c[hop]  # [G, M]
        pc = mr_h[:, 0::2] * R + mr_h[:, 1::2]  # [G, NPAIR]
        for j in range(TPH):
            blk = pc[j * 128 : (j + 1) * 128]  # [128, NPAIR]
            ri_host[hop, j] = _wrap16(blk.T.reshape(-1))  # pos i = c*128+p

    gi_flat = np.ascontiguousarray(gi_host.transpose(2, 0, 1, 3).reshape(128, -1))
    si_flat = np.ascontiguousarray(si_host.transpose(2, 0, 1, 3).reshape(128, -1))
    ri_flat = np.ascontiguousarray(ri_host.transpose(2, 0, 1, 3).reshape(128, -1))
    return gi_flat, si_flat, ri_flat


def make_in_maps(**inputs):
    ent, relpair = _prep_shared(inputs["entity_emb"], inputs["relation_emb"])
    items32 = np.asarray(inputs["items"], dtype=np.int32)
    mh_all = np.asarray(inputs["memories_h"], dtype=np.int32)
    mr_all = np.asarray(inputs["memories_r"], dtype=np.int32)
    mt_all = np.asarray(inputs["memories_t"], dtype=np.int32)

    in_maps = []
    for c in range(NCORES):
        bsl = slice(c * BL, (c + 1) * BL)
        mh_c = mh_all[:, bsl].reshape(HOPS, G, M)
        mr_c = mr_all[:, bsl].reshape(HOPS, G, M)
        mt_c = mt_all[:, bsl].reshape(HOPS, G, M)
        gi_host, si_host, ri_host = _prep_core(items32[bsl], mh_c, mr_c, mt_c)
        in_maps.append(
            {
                "entity": ent,
                "relpair": relpair,
                "gi": gi_host,
                "si": si_host,
                "ri": ri_host,
            }
        )
    return in_maps


def assemble_output(per_core_outs):
    full = np.zeros((HOPS, B, R, D), np.float32)
    for c in range(NCORES):
        o = np.asarray(per_core_outs[c]).reshape(HOPS, G, D)  # g = j*128+p
        full[:, c * BL : (c + 1) * BL] = o.reshape(HOPS, BL, R, D)
    return full


def run_on_cores(in_maps, trace=False):
    from concourse.bass_utils import run_bass_kernel_spmd

    nc = _get_nc()
    return run_bass_kernel_spmd(
        nc, in_maps, core_ids=list(range(NCORES)), trace=trace
    )


def kernel(**inputs):
    in_maps = make_in_maps(**inputs)
    res = run_on_cores(in_maps, trace=False)
    return assemble_output([r["out"] for r in res.results])
